# revision 1
# baseline (speedup 1.0000x reference)
"""Trainium2 Bass kernel for nn_Bottleneck_11416023073044 (RFAConv bottleneck).

Sharding: pure data parallelism - 1 batch sample per NeuronCore (8 cores).

Per-core pipeline (channel-major layouts, [partition, spatial] tiles):
  cv1:    hp = silu(a1*(W1 @ x) + c1)           bf16 matmuls, ACT Silu
          written into a zero-padded 82x82 bf16 frame (SBUF)
  hp9d:   9 shifted DRAM copies of the frame (per-super-chunk windows)
  strips: per channel group g (14 ch x 9 patch-idx = 126 partitions,
          n-major interleave: partition = i*ncg + cl), one gather DMA
          builds hp9_g[(i,cl), y, x] = hp[c, y+dy, x+dx]
  z:      block-pattern matmul  pz = zb_g^T @ hp9      (bf16, PSUM fp32)
  e:      e9 = exp(pz + cg9)                           ACT -> bf16
  D:      pd[14g:..] = ones^T @ e9                     (PSUM, all groups)
  rcp:    rcp = 1/pd                                   DVE reciprocal -> bf16
  rep:    rep_g = DMA stride-0 gather broadcast of rcp rows (via DRAM)
  q:      q2 = (hp9 * e9) * rep_g                      DVE bf16 2x
  out:    po[h] += wc9_g^T @ q2  over groups           bf16 matmuls
  final:  out = x + relu(a2*po + c2)                   ACT + DVE, DMA out

All elementwise work is batched over 2 PSUM banks ([*, 2, 512] fp32 tiles,
400 valid cols each = 10 frame rows per chunk) to amortize fixed overheads.
DMA queues: split between SP and Pool (gpsimd) engines.
"""
import numpy as np
import ml_dtypes

EPS = 1e-5
B, C1, C2, H, W = 8, 256, 256, 80, 80
C_ = C2 // 2          # 128
NG = 10               # channel groups
GC = 14               # channels per group (last group has 2)
HP = H + 2            # 82
S = H * W             # 6400
ROWS_BK = 5           # frame rows per PSUM bank (400 cols)
CSB = ROWS_BK * W     # 400
CH_ROWS = 2 * ROWS_BK # rows per compute chunk (2 banks)
CS = CH_ROWS * W      # 800 columns per chunk
NCH = H // CH_ROWS    # 8 chunks
SC_ROWS = 40          # rows per strip super-chunk
NSC = H // SC_ROWS    # 2 super-chunks
FW = 80 * HP          # 6560: flat window length per shifted copy
SCW = SC_ROWS * HP    # 3280: strip window per super-chunk


def _grp(g):
    c0 = g * GC
    ncg = min(GC, C_ - c0)
    return c0, ncg, 9 * ncg


def _fold_constants(W1, g1, b1, m1, v1, Wg, bg, gg, bgw, mg, vg, Wc, bc, g2, b2,
                    m2, v2):
    """Fold BN affines and build the interleaved-layout stationaries.

    Interleave (n-major): partition k = i*ncg + cl for patch index i,
    channel-in-group cl.  Same for the output index m = n*ncg + cl.
    """
    f32 = np.float32
    bf16 = ml_dtypes.bfloat16
    cst = {}
    a1 = (g1 / np.sqrt(v1 + EPS)).astype(f32)
    c1 = (b1 - m1 * a1).astype(f32)
    cst['a1c1'] = np.stack([a1, c1], axis=1)                  # [128, 2] f32

    w1t = np.zeros((C_, 2, C_), f32)                          # [c_in, t, o]
    for t in range(2):
        w1t[:, t, :] = W1[:, t * C_:(t + 1) * C_].T
    cst['w1t'] = w1t.astype(bf16)

    ag = gg / np.sqrt(vg + EPS)                               # [128, 9]
    A = (ag[:, :, None] * Wg).astype(f32)                     # [c, n, i]
    cg = (ag * (bg - mg) + bgw).astype(f32)                   # [128, 9]

    zb = np.zeros((126, NG, 126), f32)
    ones_c = np.zeros((126, 2, GC), f32)    # variant 0: ncg=14, 1: ncg=2
    cg9 = np.zeros((126, NG), f32)
    wc9 = np.zeros((126, NG, C2), f32)
    for g in range(NG):
        c0, ncg, P = _grp(g)
        v = 0 if ncg == GC else 1
        for cl in range(ncg):
            c = c0 + cl
            for n in range(9):
                m = n * ncg + cl
                cg9[m, g] = cg[c, n]
                wc9[m, g, :] = Wc[:, c, n]
                for i in range(9):
                    zb[i * ncg + cl, g, m] = A[c, n, i]
        if g in (0, NG - 1):
            for cl in range(ncg):
                for i in range(9):
                    ones_c[i * ncg + cl, v, cl] = 1.0
    # D-round lhsT for the first group of each 4-group round: group pattern in
    # cols 0..ncg, zeros at later groups' valid cols, ones filler elsewhere
    # (keeps unused partitions of the D PSUM tile finite for reciprocal).
    onesr = np.zeros((126, 3, C_), f32)
    for r in range(3):
        grs = [g for g in range(4 * r, min(4 * r + 4, NG))]
        valid = set()
        for m, g in enumerate(grs):
            _, ncg, _ = _grp(g)
            for cl in range(ncg):
                valid.add(32 * m + cl)
        for j in range(C_):
            if j not in valid:
                onesr[:, r, j] = 1.0
        g0 = grs[0]
        _, ncg0, _ = _grp(g0)
        for cl in range(ncg0):
            for i in range(9):
                onesr[i * ncg0 + cl, r, cl] = 1.0
    cst['zb'] = zb.astype(bf16)
    cst['ones_c'] = ones_c.astype(bf16)
    cst['onesr'] = onesr.astype(bf16)
    cst['cg9'] = cg9
    cst['wc9'] = wc9.astype(bf16)

    a2 = (g2 / np.sqrt(v2 + EPS)).astype(f32)
    c2 = (b2 + a2 * (bc - m2)).astype(f32)
    a2c2 = np.zeros((C_, 2, 2), f32)
    for h in range(2):
        a2c2[:, h, 0] = a2[h * C_:(h + 1) * C_]
        a2c2[:, h, 1] = c2[h * C_:(h + 1) * C_]
    cst['a2c2'] = a2c2
    return cst


_PROGRAM = None


def _build_program():
    import concourse.bass as bass
    import concourse.tile as tile
    from concourse import mybir

    dt = mybir.dt
    AF = mybir.ActivationFunctionType

    nc = bass.Bass("TRN2", target_bir_lowering=False, debug=False)

    xs_d = nc.dram_tensor("xs", [C_, 2, S], dt.bfloat16, kind="ExternalInput")
    w1t_d = nc.dram_tensor("w1t", [C_, 2, C_], dt.bfloat16, kind="ExternalInput")
    a1c1_d = nc.dram_tensor("a1c1", [C_, 2], dt.float32, kind="ExternalInput")
    zb_d = nc.dram_tensor("zb", [126, NG, 126], dt.bfloat16, kind="ExternalInput")
    ones_d = nc.dram_tensor("ones_c", [126, 2, GC], dt.bfloat16, kind="ExternalInput")
    onesr_d = nc.dram_tensor("onesr", [126, 3, C_], dt.bfloat16, kind="ExternalInput")
    cg9_d = nc.dram_tensor("cg9", [126, NG], dt.float32, kind="ExternalInput")
    wc9_d = nc.dram_tensor("wc9", [126, NG, C2], dt.bfloat16, kind="ExternalInput")
    a2c2_d = nc.dram_tensor("a2c2", [C_, 2, 2], dt.float32, kind="ExternalInput")
    out_d = nc.dram_tensor("out", [C_, 2, S], dt.bfloat16, kind="ExternalOutput")
    hp9d = nc.dram_tensor("hp9d", [9, C_, FW], dt.bfloat16)
    rcp_d = nc.dram_tensor("rcp_d", [NCH, 3, C_, CS], dt.bfloat16)

    with tile.TileContext(nc) as tc:
        with tc.tile_pool(name="singles", bufs=1) as singles, \
             tc.tile_pool(name="strips", bufs=1) as strips, \
             tc.tile_pool(name="work", bufs=3) as work, \
             tc.tile_pool(name="qpool", bufs=2) as qpool, \
             tc.tile_pool(name="psz", bufs=2, space="PSUM") as psz, \
             tc.tile_pool(name="psd", bufs=1, space="PSUM") as psd, \
             tc.tile_pool(name="pso", bufs=1, space="PSUM") as pso:

            # ---- resident tiles + constant loads ----
            # critical path first on SP: w1t, a1c1, then x halves
            w1t = singles.tile([C_, 2, C_], dt.bfloat16, tag="w1t", name="w1t")
            nc.sync.dma_start(out=w1t[:], in_=w1t_d[:])
            a1c1 = singles.tile([C_, 2], dt.float32, tag="a1c1", name="a1c1")
            nc.sync.dma_start(out=a1c1[:], in_=a1c1_d[:])
            x2 = singles.tile([C_, 2, S], dt.bfloat16, tag="x2", name="x2")
            for qtr in range(8):
                cols = slice(qtr * (S // 8), (qtr + 1) * (S // 8))
                for t in range(2):
                    nc.sync.dma_start(out=x2[:, t, cols], in_=xs_d[:, t, cols])
            # main-loop stationaries on the Pool queue (idle at the front)
            zb = singles.tile([126, NG, 126], dt.bfloat16, tag="zb", name="zb")
            nc.gpsimd.dma_start(out=zb[:], in_=zb_d[:])
            ones_c = singles.tile([126, 2, GC], dt.bfloat16, tag="ones_c", name="ones_c")
            nc.gpsimd.dma_start(out=ones_c[:], in_=ones_d[:])
            onesr = singles.tile([126, 3, C_], dt.bfloat16, tag="onesr", name="onesr")
            nc.gpsimd.dma_start(out=onesr[:], in_=onesr_d[:])
            cg9 = singles.tile([126, NG], dt.float32, tag="cg9", name="cg9")
            nc.gpsimd.dma_start(out=cg9[:], in_=cg9_d[:])
            wc9 = singles.tile([126, NG, C2], dt.bfloat16, tag="wc9", name="wc9")
            nc.gpsimd.dma_start(out=wc9[:], in_=wc9_d[:])
            a2c2 = singles.tile([C_, 2, 2], dt.float32, tag="a2c2", name="a2c2")
            nc.gpsimd.dma_start(out=a2c2[:], in_=a2c2_d[:])

            hpfl = singles.tile([C_, HP * HP + 2], dt.bfloat16, tag="hp", name="hp")
            hp = hpfl[:, 0:HP * HP].rearrange("p (a b) -> p a b", a=HP)
            # zero only the pad region: top row, bottom row, left/right cols, tail
            nc.vector.memset(hp[:, 0, :], 0.0)
            nc.vector.memset(hp[:, HP - 1, :], 0.0)
            nc.vector.memset(hp[:, 1:HP - 1, 0], 0.0)
            nc.vector.memset(hp[:, 1:HP - 1, HP - 1], 0.0)
            nc.vector.memset(hpfl[:, HP * HP:], 0.0)

            # ---- phase A/B/C interleave: cv1 chunks, then per-sc copy
            #      windows + strip gathers as soon as their rows are ready ----
            def cv1_chunk(ch):
                y0 = ch * CH_ROWS
                ph = psz.tile([C_, 2, 512], dt.float32, tag="pz", name="ph")
                for k in range(2):
                    cols = slice((2 * ch + k) * CSB, (2 * ch + k + 1) * CSB)
                    for t in range(2):
                        nc.tensor.matmul(
                            out=ph[:, k, 0:CSB],
                            lhsT=w1t[:, t, :],
                            rhs=x2[:, t, cols],
                            start=(t == 0), stop=(t == 1))
                yb = work.tile([C_, 2, CSB], dt.bfloat16, tag="yb", name="yb")
                nc.vector.tensor_scalar(yb[:], ph[:, :, 0:CSB],
                                        a1c1[:, 0:1], a1c1[:, 1:2],
                                        mybir.AluOpType.mult,
                                        mybir.AluOpType.add)
                sg = work.tile([C_, 2, CSB], dt.bfloat16, tag="sg", name="sg")
                nc.scalar.activation(out=sg[:], in_=ph[:, :, 0:CSB],
                                     func=AF.Sigmoid,
                                     scale=a1c1[:, 0:1], bias=a1c1[:, 1:2])
                nc.vector.tensor_mul(hp[:, 1 + y0:1 + y0 + CH_ROWS, 1:1 + W],
                                     yb[:], sg[:])

            def strip_gathers(sc):
                ys = sc * SC_ROWS
                hp9 = []
                for g in range(NG):
                    c0, ncg, P = _grp(g)
                    st = strips.tile([126, SC_ROWS, HP], dt.bfloat16,
                                     tag=f"hp9_{g}", name=f"hp9_{g}")
                    hp9.append(st)
                    srcap = bass.AP(
                        tensor=hp9d[0].tensor, offset=c0 * FW + ys * HP,
                        ap=[[C_ * FW, 9], [FW, ncg], [1, SCW]])
                    eng = nc.gpsimd if g % 2 == 0 else nc.sync
                    eng.dma_start(out=st[0:P, :, :], in_=srcap)
                return hp9

            def hp9d_incr(ch):
                # write copy cols [lo, hi): needs hpfl through 820*ch+902,
                # i.e. cv1 chunk ch done; issued 2 chunks later so the
                # whole-tile WAR on hpfl never stalls cv1.
                lo = 0 if ch == 0 else 820 * ch - 84
                hi = 820 * ch + 736 if ch < NCH - 1 else FW
                engs = [nc.sync, nc.gpsimd, nc.scalar]
                for i in range(9):
                    d = (i // 3) * HP + (i % 3)
                    engs[i % 3].dma_start(
                        out=hp9d[i][:, lo:hi],
                        in_=hpfl[:, d + lo:d + hi])

            for ch in range(NCH):
                cv1_chunk(ch)
                if ch >= 2:
                    hp9d_incr(ch - 2)
                if ch == 6:
                    hp9_cur = strip_gathers(0)
            hp9d_incr(NCH - 2)
            hp9d_incr(NCH - 1)

            # ==== per super-chunk: main ====
            for sc in range(NSC):
                if sc > 0:
                    hp9_cur = strip_gathers(sc)
                hp9 = hp9_cur

                # ---- phase D: main loop over chunks of this super-chunk ----
                for cc in range(SC_ROWS // CH_ROWS):
                    ch = sc * (SC_ROWS // CH_ROWS) + cc
                    pd = None
                    q1s = []
                    q2s = []
                    for g in range(NG):
                        c0, ncg, P = _grp(g)
                        v = 0 if ncg == GC else 1
                        r, m = divmod(g, 4)
                        last_in_round = (g == NG - 1) or (m == 3)
                        pz = psz.tile([C_, 2, 512], dt.float32, tag="pz", name="pz")
                        for k in range(2):
                            rows = slice((2 * cc + k) * ROWS_BK,
                                         (2 * cc + k + 1) * ROWS_BK)
                            nc.tensor.matmul(
                                out=pz[0:P, k, 0:CSB],
                                lhsT=zb[0:P, g, 0:P],
                                rhs=hp9[g][0:P, rows, 0:W],
                                start=True, stop=True)
                        e9 = work.tile([126, 2, CSB], dt.bfloat16, tag="e9", name="e9")
                        nc.scalar.activation(out=e9[0:P, :, :],
                                             in_=pz[0:P, :, 0:CSB],
                                             func=AF.Exp, bias=cg9[0:P, g:g + 1])
                        rows10 = slice(2 * cc * ROWS_BK, (2 * cc + 2) * ROWS_BK)
                        q1 = qpool.tile([126, 2, CSB], dt.bfloat16,
                                        tag=f"q1_{g}", name=f"q1_{g}", bufs=1)
                        q1s.append(q1)
                        nc.vector.tensor_mul(q1[0:P, :, :],
                                             hp9[g][0:P, rows10, 0:W],
                                             e9[0:P, :, :])
                        if m == 0:
                            pd = psd.tile([C_, 2, 512], dt.float32, tag="pd",
                                          name="pd")
                        for k in range(2):
                            if m == 0:
                                nc.tensor.matmul(
                                    out=pd[:, k, 0:CSB],
                                    lhsT=onesr[0:P, r, :],
                                    rhs=e9[0:P, k, :],
                                    start=True, stop=last_in_round,
                                    skip_group_check=True)
                            else:
                                nc.tensor.matmul(
                                    out=pd[32 * m:32 * m + ncg, k, 0:CSB],
                                    lhsT=ones_c[0:P, v, 0:ncg],
                                    rhs=e9[0:P, k, :],
                                    start=False, stop=last_in_round,
                                    skip_group_check=True,
                                    tile_position=(0, 32 * m))
                        if last_in_round:
                            rcp = work.tile([C_, 2, CSB], dt.bfloat16,
                                            tag="rcp", name="rcp")
                            with nc.allow_low_precision("softmax denom bf16"):
                                nc.vector.reciprocal(rcp[:], pd[:, :, 0:CSB])
                            nc.sync.dma_start(
                                out=rcp_d[ch, r],
                                in_=rcp[:].rearrange("p a b -> p (a b)"))
                            for gg in range(4 * r, g + 1):
                                cg0, ncgg, Pg = _grp(gg)
                                mm = gg - 4 * r
                                rep = work.tile([126, 2, CSB], dt.bfloat16,
                                                tag="rep", name="rep")
                                repsrc = bass.AP(
                                    tensor=rcp_d[0].tensor,
                                    offset=(ch * 3 + r) * (C_ * CS) + (32 * mm) * CS,
                                    ap=[[0, 9], [CS, ncgg], [1, CS]])
                                eng = nc.gpsimd if gg % 2 == 1 else nc.sync
                                eng.dma_start(
                                    out=rep[0:Pg, :, :].rearrange("p a b -> p (a b)"),
                                    in_=repsrc)
                                q2 = qpool.tile([126, 2, CSB], dt.bfloat16,
                                                tag=f"q2_{gg}", name=f"q2_{gg}",
                                                bufs=1)
                                q2s.append(q2)
                                nc.vector.tensor_mul(q2[0:Pg, :, :],
                                                     q1s[gg][0:Pg, :, :],
                                                     rep[0:Pg, :, :])

                    for h in range(2):
                        po = pso.tile([C_, 2, 512], dt.float32, tag="po", name="po")
                        for g in range(NG):
                            c0, ncg, P = _grp(g)
                            for k in range(2):
                                nc.tensor.matmul(
                                    out=po[:, k, 0:CSB],
                                    lhsT=wc9[0:P, g, h * C_:(h + 1) * C_],
                                    rhs=q2s[g][0:P, k, :],
                                    start=(g == 0), stop=(g == NG - 1))
                        t = work.tile([C_, 2, CSB], dt.bfloat16, tag=f"t{h}", name=f"t{h}")
                        nc.scalar.activation(out=t[:], in_=po[:, :, 0:CSB],
                                             func=AF.Relu,
                                             scale=a2c2[:, h, 0:1],
                                             bias=a2c2[:, h, 1:2])
                        og = work.tile([C_, 2, CSB], dt.bfloat16, tag=f"og{h}", name=f"og{h}")
                        adde = nc.vector if ch == NCH - 1 else nc.gpsimd
                        adde.tensor_add(og[:], t[:],
                                        x2[:, h, ch * CS:(ch + 1) * CS])
                        oeng = nc.gpsimd if (ch == NCH - 1 and h == 1) else nc.sync
                        oeng.dma_start(
                            out=out_d[:, h, ch * CS:(ch + 1) * CS],
                            in_=og[:].rearrange("p a b -> p (a b)"))

    _split_excess_waits(nc)
    return nc


def _split_excess_waits(nc):
    """This walrus build rejects >1 sync-wait on TPB_CTRL instructions and
    >2 elsewhere; redistribute onto same-engine wait-nops inserted before."""
    import concourse.mybir as mybir
    cnt = [0]
    for bb in nc.main_func.blocks:
        new_list = []
        changed = False
        for ins in bb.instructions:
            si = ins.sync_info
            lim = 1
            if si is not None and si.on_wait is not None and len(si.on_wait) > lim:
                waits = list(si.on_wait)
                head, tail = waits[:-lim], waits[-lim:]
                for w in head:
                    nop = mybir.InstNoOp(name=f"waitsplit-{cnt[0]}", ins=[], outs=[])
                    cnt[0] += 1
                    nop.engine = ins.engine
                    nop.sync_info = mybir.SyncInfo(on_wait=[w], on_update=[])
                    nop.bass_nofuse = True
                    try:
                        nc.register_instruction(nop)
                    except Exception:
                        pass
                    new_list.append(nop)
                ins.sync_info = mybir.SyncInfo(
                    on_wait=tail, on_update=list(si.on_update or []))
                changed = True
            new_list.append(ins)
        if changed:
            bb.instructions[:] = new_list


def _get_program():
    global _PROGRAM
    if _PROGRAM is None:
        _PROGRAM = _build_program()
    return _PROGRAM


def _pack_inputs(x_b):
    """x_b: [C1, H, W] fp32 -> xs [128, 2, 6400] bf16."""
    bf16 = ml_dtypes.bfloat16
    xr = x_b.reshape(2, C_, S)            # [t, c, s]
    return np.ascontiguousarray(xr.transpose(1, 0, 2)).astype(bf16)


def kernel(**inputs):
    from concourse.bass_utils import run_bass_kernel_spmd

    x = np.asarray(inputs['x'], dtype=np.float32)
    cst = _fold_constants(**{k: np.asarray(v, dtype=np.float32)
                             for k, v in inputs.items() if k != 'x'})
    nc = _get_program()
    base = {
        'w1t': cst['w1t'], 'a1c1': cst['a1c1'], 'zb': cst['zb'],
        'ones_c': cst['ones_c'], 'onesr': cst['onesr'], 'cg9': cst['cg9'],
        'wc9': cst['wc9'], 'a2c2': cst['a2c2'],
    }
    in_maps = [dict(base, xs=_pack_inputs(x[b].reshape(C1, H * W)))
               for b in range(B)]
    res = run_bass_kernel_spmd(nc, in_maps, list(range(B)))
    out = np.empty((B, C2, H, W), dtype=np.float32)
    for b in range(B):
        ob = res.results[b]['out'].astype(np.float32)     # [128, 2, 6400]
        out[b] = ob.transpose(1, 0, 2).reshape(C2, H, W)
    return out



# revision 28
# speedup vs baseline: 1.1708x; 1.1708x over previous
"""Trainium2 Bass kernel for nn_Bottleneck_11416023073044 (RFAConv bottleneck).

Sharding: pure data parallelism - 1 batch sample per NeuronCore (8 cores).

Per-core pipeline (channel-major layouts, [partition, spatial] tiles).
Partition interleave is cl-major: p = cl*9 + n for channel-in-group cl and
patch index n.  Groups of GC=14 channels (last group has 2).

  cv1:    ph = W1' @ x  (W1 pre-scaled by BN a1), PSUM
          sg = Sigmoid(ph + c1)       ACT, bias=c1
          hp = (ph + c1) * sg         DVE scalar_tensor_tensor -> bf16 frame
  hp9d:   3 DMAs per chunk write the 9 shifted flat windows (grouped by
          row-shift a, the 3 col-shifts b are one AP dim) to DRAM
  strips: per block (20 rows) per group g: one gather DMA builds
          st_g[(cl,n), r, x] = hp[c, (ys+r)*82 + x + d_n]
  z:      pz = zb_g^T @ st            (bf16, PSUM fp32)
  e:      e9 = exp(pz + cg9)          ACT -> bf16 (2-chunk buffer)
  D:      pd = sum over groups of dones_g^T @ e9_g  (one PSUM accumulation
          over all 10 groups; output partition = channel)
  rcp:    rcp2[ch] = 1/pd             DVE reciprocal -> bf16
  q1:     e9q1_g *= st_g              DVE in-place (patches * e)
  rep:    rep_g = SBUF->SBUF DMA broadcast of rcp2 rows (0-stride over n)
  q2:     q2_g = e9q1_g * rep_g       DVE
  out:    po[h] += wc9_g^T @ q2_g     bf16 matmuls over groups
  final:  t = relu(a2*po + c2) ACT;  og = x + t (Pool);  DMA out
"""
import numpy as np
import ml_dtypes

EPS = 1e-5
B, C1, C2, H, W = 8, 256, 256, 80, 80
C_ = C2 // 2          # 128
NG = 10               # channel groups
GC = 14               # channels per group (last group has 2)
HP = H + 2            # 82
S = H * W             # 6400
ROWS_BK = 5           # frame rows per PSUM bank (400 cols)
CSB = ROWS_BK * W     # 400
CH_ROWS = 2 * ROWS_BK # rows per compute chunk (2 banks)
CS = CH_ROWS * W      # 800 columns per chunk
NCH = H // CH_ROWS    # 8 chunks
BLK_ROWS = 20         # rows per block (2 chunks)
NBLK = H // BLK_ROWS  # 4 blocks
BW = BLK_ROWS * HP    # 1640: strip window per block
FW = 80 * HP          # 6560: flat window length per shifted copy
HPF = HP * HP + 2     # 6726 flat frame length (tail padded)


def _grp(g):
    c0 = g * GC
    ncg = min(GC, C_ - c0)
    return c0, ncg, 9 * ncg


def _fold_constants(W1, g1, b1, m1, v1, Wg, bg, gg, bgw, mg, vg, Wc, bc, g2, b2,
                    m2, v2):
    """Fold BN affines and build the cl-major-layout stationaries.

    Partition index p = cl*9 + n for channel-in-group cl, patch index n.
    """
    f32 = np.float32
    bf16 = ml_dtypes.bfloat16
    cst = {}
    a1 = (g1 / np.sqrt(v1 + EPS)).astype(f32)
    c1 = (b1 - m1 * a1).astype(f32)
    cst['c1sig'] = c1.reshape(C_, 1)

    w1t = np.zeros((C_, 2, C_), f32)                          # [c_in, t, o]
    for t in range(2):
        w1t[:, t, :] = (a1[:, None] * W1[:, t * C_:(t + 1) * C_]).T
    cst['w1t'] = w1t.astype(bf16)

    ag = gg / np.sqrt(vg + EPS)                               # [128, 9]
    A = (ag[:, :, None] * Wg).astype(f32)                     # [c, n, i]
    cg = (ag * (bg - mg) + bgw).astype(f32)                   # [128, 9]

    zb = np.zeros((126, NG, 126), f32)
    cg9 = np.zeros((126, NG), f32)
    wc9 = np.zeros((126, NG, C2), f32)
    dones = np.zeros((126, NG, C_), f32)
    for g in range(NG):
        c0, ncg, P = _grp(g)
        for cl in range(ncg):
            c = c0 + cl
            for n in range(9):
                m = cl * 9 + n
                cg9[m, g] = cg[c, n]
                wc9[m, g, :] = Wc[:, c, n]
                dones[m, g, c] = 1.0
                for i in range(9):
                    zb[cl * 9 + i, g, m] = A[c, n, i]
    cst['zb'] = zb.astype(bf16)
    cst['cg9'] = cg9
    cst['wc9'] = wc9.astype(bf16)
    cst['dones'] = dones.astype(bf16)

    a2 = (g2 / np.sqrt(v2 + EPS)).astype(f32)
    c2 = (b2 + a2 * (bc - m2)).astype(f32)
    a2c2 = np.zeros((C_, 2, 2), f32)
    for h in range(2):
        a2c2[:, h, 0] = a2[h * C_:(h + 1) * C_]
        a2c2[:, h, 1] = c2[h * C_:(h + 1) * C_]
    cst['a2c2'] = a2c2
    return cst


_PROGRAM = None


def _build_program():
    import concourse.bass as bass
    import concourse.tile as tile
    from concourse import mybir

    dt = mybir.dt
    AF = mybir.ActivationFunctionType
    ALU = mybir.AluOpType

    nc = bass.Bass("TRN2", target_bir_lowering=False, debug=False)

    xs_d = nc.dram_tensor("xs", [C_, 2, S], dt.bfloat16, kind="ExternalInput")
    w1t_d = nc.dram_tensor("w1t", [C_, 2, C_], dt.bfloat16, kind="ExternalInput")
    c1_d = nc.dram_tensor("c1sig", [C_, 1], dt.float32, kind="ExternalInput")
    zb_d = nc.dram_tensor("zb", [126, NG, 126], dt.bfloat16, kind="ExternalInput")
    dones_d = nc.dram_tensor("dones", [126, NG, C_], dt.bfloat16, kind="ExternalInput")
    cg9_d = nc.dram_tensor("cg9", [126, NG], dt.float32, kind="ExternalInput")
    wc9_d = nc.dram_tensor("wc9", [126, NG, C2], dt.bfloat16, kind="ExternalInput")
    a2c2_d = nc.dram_tensor("a2c2", [C_, 2, 2], dt.float32, kind="ExternalInput")
    out_d = nc.dram_tensor("out", [C_, 2, S], dt.bfloat16, kind="ExternalOutput")
    hp9d = nc.dram_tensor("hp9d", [9, C_, FW], dt.bfloat16)

    with tile.TileContext(nc) as tc:
        with tc.tile_pool(name="singles", bufs=1) as singles, \
             tc.tile_pool(name="strips", bufs=1) as strips, \
             tc.tile_pool(name="eq", bufs=1) as eqpool, \
             tc.tile_pool(name="q2p", bufs=1) as q2pool, \
             tc.tile_pool(name="work", bufs=3) as work, \
             tc.tile_pool(name="repp", bufs=6) as repp, \
             tc.tile_pool(name="rcpp", bufs=2) as rcpp, \
             tc.tile_pool(name="ogp", bufs=2) as ogp, \
             tc.tile_pool(name="psz", bufs=2, space="PSUM") as psz, \
             tc.tile_pool(name="psd", bufs=1, space="PSUM") as psd, \
             tc.tile_pool(name="pso", bufs=1, space="PSUM") as pso:

            # ---- resident tiles + constant loads ----
            # SP front-loads the cv1 critical path chunk by chunk
            x2 = singles.tile([C_, 2, S], dt.bfloat16, tag="x2", name="x2")
            nc.sync.dma_start(out=x2[:, :, 0:CS], in_=xs_d[:, :, 0:CS])
            w1t = singles.tile([C_, 2, C_], dt.bfloat16, tag="w1t", name="w1t")
            nc.sync.dma_start(out=w1t[:], in_=w1t_d[:])
            c1sig = singles.tile([C_, 1], dt.float32, tag="c1sig", name="c1sig")
            nc.gpsimd.dma_start(out=c1sig[:], in_=c1_d[:])
            atl = singles.tile([1, 2], dt.float32, tag="atl", name="atl")
            nc.scalar.activation(out=atl[0:1, 0:1], in_=c1sig[0:1, 0:1],
                                 func=AF.Sigmoid)
            nc.sync.dma_start(out=x2[:, :, CS:2 * CS], in_=xs_d[:, :, CS:2 * CS])
            zb = singles.tile([126, NG, 126], dt.bfloat16, tag="zb", name="zb")
            nc.gpsimd.dma_start(out=zb[:], in_=zb_d[:])
            nc.sync.dma_start(out=x2[:, :, 2 * CS:4 * CS],
                              in_=xs_d[:, :, 2 * CS:4 * CS])
            nc.gpsimd.dma_start(out=x2[:, :, 4 * CS:6 * CS],
                                in_=xs_d[:, :, 4 * CS:6 * CS])
            nc.gpsimd.dma_start(out=x2[:, :, 6 * CS:S],
                                in_=xs_d[:, :, 6 * CS:S])
            cg9 = singles.tile([126, NG], dt.float32, tag="cg9", name="cg9")
            nc.gpsimd.dma_start(out=cg9[:], in_=cg9_d[:])
            dones = singles.tile([126, NG, C_], dt.bfloat16, tag="dones", name="dones")
            nc.gpsimd.dma_start(out=dones[:], in_=dones_d[:])
            wc9 = singles.tile([126, NG, C2], dt.bfloat16, tag="wc9", name="wc9")
            a2c2 = singles.tile([C_, 2, 2], dt.float32, tag="a2c2", name="a2c2")

            hpfl = singles.tile([C_, HPF], dt.bfloat16, tag="hp", name="hp")
            hp = hpfl[:, 0:HP * HP].rearrange("p (a b) -> p a b", a=HP)
            nc.vector.memset(hp[:, 0, :], 0.0)
            nc.vector.memset(hp[:, HP - 1, :], 0.0)
            nc.vector.memset(hp[:, 1:HP - 1, 0], 0.0)
            nc.vector.memset(hp[:, 1:HP - 1, HP - 1], 0.0)
            nc.vector.memset(hpfl[:, HP * HP:], 0.0)

            # per-group tiles (allocated up front, bufs=1 semantics)
            st = []
            eq = []
            q2 = []
            for g in range(NG):
                c0, ncg, P = _grp(g)
                st.append(strips.tile([P, BLK_ROWS, HP], dt.bfloat16,
                                      tag=f"st{g}", name=f"st{g}"))
                eq.append(eqpool.tile([P, 2, 2, CSB], dt.bfloat16,
                                      tag=f"eq{g}", name=f"eq{g}"))
                q2.append(q2pool.tile([P, 2, 2, CSB], dt.bfloat16,
                                      tag=f"q2{g}", name=f"q2{g}"))

            # ---- phase A: cv1 chunks + incremental hp9d writes ----
            def cv1_chunk(ch, pool=None, tag="pz"):
                y0 = ch * CH_ROWS
                ph = (pool or psz).tile([C_, 2, 512], dt.float32, tag=tag,
                                        name="ph")
                for k in range(2):
                    cols = slice((2 * ch + k) * CSB, (2 * ch + k + 1) * CSB)
                    for t in range(2):
                        nc.tensor.matmul(
                            out=ph[:, k, 0:CSB],
                            lhsT=w1t[:, t, :],
                            rhs=x2[:, t, cols],
                            start=(t == 0), stop=(t == 1))
                sg = work.tile([C_, 2, CSB], dt.bfloat16, tag="sg", name="sg")
                nc.scalar.activation(out=sg[:], in_=ph[:, :, 0:CSB],
                                     func=AF.Sigmoid, bias=c1sig[:, 0:1])
                nc.vector.scalar_tensor_tensor(
                    out=hp[:, 1 + y0:1 + y0 + CH_ROWS, 1:1 + W],
                    in0=ph[:, :, 0:CSB], scalar=c1sig[:, 0:1], in1=sg[:],
                    op0=ALU.add, op1=ALU.mult)
                return sg

            def hp9d_incr(ch):
                # write copy cols [lo, hi): 3 DMAs, one per row-shift a, each
                # on its own queue; the col-shifts b ride as an AP dim.
                lo = 0 if ch == 0 else 820 * ch - 84
                hi = 820 * ch + 736 if ch < NCH - 1 else FW
                ln = hi - lo
                wengs = [nc.sync, nc.gpsimd,
                         nc.sync if ch % 2 == 0 else nc.gpsimd]
                for a in range(3):
                    dst = bass.AP(tensor=hp9d[0].tensor,
                                  offset=(3 * a) * (C_ * FW) + lo,
                                  ap=[[FW, C_], [C_ * FW, 3], [1, ln]])
                    src = bass.AP(tensor=hpfl[:].tensor,
                                  offset=a * HP + lo,
                                  ap=[[HPF, C_], [1, 3], [1, ln]])
                    wengs[a].dma_start(out=dst, in_=src)

            def strip_gather(b, g, eng, r0=0, r1=BLK_ROWS):
                ys = b * BLK_ROWS + r0
                c0, ncg, P = _grp(g)
                ln = (r1 - r0) * HP
                srcap = bass.AP(
                    tensor=hp9d[0].tensor, offset=c0 * FW + ys * HP,
                    ap=[[FW, ncg], [C_ * FW, 9], [1, ln]])
                eng.dma_start(
                    out=st[g][:, r0:r1, :].rearrange("p a b -> p (a b)"),
                    in_=srcap)

            cvpool = {2: (pso, "po"), 3: (psd, "pd"), 6: (pso, "po"),
                      7: (psd, "pd")}
            sg_last = None
            for ch in range(NCH):
                pl, tg = cvpool.get(ch, (None, "pz"))
                sg_last = cv1_chunk(ch, pool=pl, tag=tg)
                hp9d_incr(ch)
                if ch == 1:
                    for g in range(NG):
                        strip_gather(0, g, [nc.scalar, nc.sync, nc.gpsimd][g % 3],
                                     0, CH_ROWS)
                if ch == 2:
                    for g in range(NG):
                        strip_gather(0, g, [nc.sync, nc.gpsimd, nc.scalar][g % 3],
                                     CH_ROWS, BLK_ROWS)
                    nc.gpsimd.dma_start(out=wc9[:], in_=wc9_d[:])
                    nc.gpsimd.dma_start(out=a2c2[:], in_=a2c2_d[:])
            nc.scalar.activation(out=atl[0:1, 1:2], in_=sg_last[0:1, 0:1, 0:1],
                                 func=AF.Exp)

            og_cur = [None]
            po_cur = [None]

            def out_mms(ch, h, j0, j1, pool=None, tag="po"):
                # a 5-matmul slice of the skewed out conv's po accumulation
                # for chunk ch, output half h (20 matmuls = 10 groups x 2
                # banks, emitted in (g, k) order)
                b, cb = divmod(ch, 2)
                if j0 == 0 and h == 0:
                    og_cur[0] = ogp.tile([C_, 2, CS], dt.bfloat16, tag="og",
                                         name="og")
                if j0 == 0:
                    po_cur[0] = (pool or pso).tile([C_, 2, 512], dt.float32,
                                                   tag=tag, name="po")
                po = po_cur[0]
                for j in range(j0, j1):
                    g, k = divmod(j, 2)
                    c0, ncg, P = _grp(g)
                    nc.tensor.matmul(
                        out=po[:, k, 0:CSB],
                        lhsT=wc9[0:P, g, h * C_:(h + 1) * C_],
                        rhs=q2[g][:, cb, k, :],
                        start=(g == 0), stop=(g == NG - 1),
                        skip_group_check=True)

            def out_epi(ch, h, adde=None):
                b, cb = divmod(ch, 2)
                og = og_cur[0]
                po = po_cur[0]
                t = work.tile([C_, 2, CSB], dt.bfloat16, tag=f"t{h}",
                              name=f"t{h}")
                nc.scalar.activation(out=t[:], in_=po[:, :, 0:CSB],
                                     func=AF.Relu,
                                     scale=a2c2[:, h, 0:1],
                                     bias=a2c2[:, h, 1:2])
                (adde or nc.gpsimd).tensor_add(
                    og[:, h, :].rearrange("p (a b) -> p a b", a=2),
                    t[:], x2[:, h, ch * CS:(ch + 1) * CS]
                    .rearrange("p (a b) -> p a b", a=2))
                if ch == NCH - 1:
                    dstap = bass.AP(tensor=out_d[0].tensor,
                                    offset=h * S + ch * CS,
                                    ap=[[2 * S, C_], [1, CS]])
                    nc.sync.dma_start(out=dstap, in_=og[:, h, :])
                elif h == 1:
                    dstap = bass.AP(tensor=out_d[0].tensor,
                                    offset=ch * CS,
                                    ap=[[2 * S, C_], [S, 2], [1, CS]])
                    nc.sync.dma_start(out=dstap, in_=og[:])

            # ---- main loop, one chunk per cycle; the out conv for chunk
            #      ch-2 runs in chunk ch's tail so its q2 operands are long
            #      since finalized ----
            rcp2 = None
            for ch in range(NCH):
                b, cb = divmod(ch, 2)
                if True:
                    if cb == 0:
                        rcp2 = rcpp.tile([C_, 2, 2, CSB], dt.bfloat16,
                                         tag="rcp2", name="rcp2")
                    pd = None
                    for g in range(NG):
                        c0, ncg, P = _grp(g)
                        pz = psz.tile([C_, 2, 512], dt.float32, tag="pz",
                                      name="pz")
                        for k in range(2):
                            rows = slice(cb * CH_ROWS + k * ROWS_BK,
                                         cb * CH_ROWS + (k + 1) * ROWS_BK)
                            nc.tensor.matmul(
                                out=pz[0:P, k, 0:CSB],
                                lhsT=zb[0:P, g, 0:P],
                                rhs=st[g][:, rows, 0:W],
                                start=True, stop=True)
                        nc.scalar.activation(out=eq[g][:, cb, :, :],
                                             in_=pz[0:P, :, 0:CSB],
                                             func=AF.Exp,
                                             bias=cg9[0:P, g:g + 1])
                        if g == 0:
                            pd = psd.tile([C_, 2, 512], dt.float32,
                                          tag="pd", name="pd")
                        for k in range(2):
                            nc.tensor.matmul(
                                out=pd[:, k, 0:CSB],
                                lhsT=dones[0:P, g, :],
                                rhs=eq[g][:, cb, k, :],
                                start=(g == 0), stop=(g == NG - 1),
                                skip_group_check=True)
                        if ch >= 2:
                            if g <= 3:
                                out_mms(ch - 2, 0, g * 5, g * 5 + 5)
                            elif g == 4:
                                out_epi(ch - 2, 0)
                            elif g <= 8:
                                out_mms(ch - 2, 1, (g - 5) * 5, (g - 5) * 5 + 5)
                            else:
                                out_epi(ch - 2, 1)
                    with nc.allow_low_precision("softmax denom bf16"):
                        nc.vector.reciprocal(rcp2[:, cb, :, :],
                                             pd[:, :, 0:CSB])
                if ch >= NCH:
                    continue

                gengs = [nc.sync, nc.gpsimd]
                rcpfl = rcp2[:].rearrange("p a b c -> p (a b c)")
                reps = []
                for g in range(NG):
                    c0, ncg, P = _grp(g)
                    rep = repp.tile([126, 2, CSB], dt.bfloat16, tag="rep",
                                    name="rep")
                    reps.append(rep)
                    repsrc = bass.AP(
                        tensor=rcpfl.tensor,
                        offset=rcpfl.offset + c0 * (4 * CSB) + cb * (2 * CSB),
                        ap=[[4 * CSB, ncg], [0, 9], [1, 2 * CSB]])
                    gengs[(g + 1) % 2].dma_start(out=rep[0:P, :, :]
                                                 .rearrange("p a b -> p (a b)"),
                                                 in_=repsrc)
                for g in range(NG):
                    c0, ncg, P = _grp(g)
                    # q1 into the q2 tile (leaves eq free for the next exp)
                    nc.vector.tensor_mul(
                        q2[g][:, cb, :, :], eq[g][:, cb, :, :],
                        st[g][:, cb * CH_ROWS:(cb + 1) * CH_ROWS, 0:W])
                    # q2 in place
                    nc.vector.tensor_mul(
                        q2[g][:, cb, :, :], q2[g][:, cb, :, :],
                        reps[g][0:P, :, :])
                    if cb == 1 and b + 1 < NBLK:
                        strip_gather(b + 1, g, gengs[g % 2])

            for i, (ch, h) in enumerate([(NCH - 2, 0), (NCH - 2, 1),
                                         (NCH - 1, 0), (NCH - 1, 1)]):
                out_mms(ch, h, 0, 20, pool=psz if i % 2 else pso,
                        tag="pz" if i % 2 else "po")
                out_epi(ch, h, adde=nc.vector)

    _split_excess_waits(nc)
    return nc


def _split_excess_waits(nc):
    """This walrus build rejects >1 sync-wait per instruction; redistribute
    onto same-engine wait-nops inserted before."""
    import concourse.mybir as mybir
    cnt = [0]
    for bb in nc.main_func.blocks:
        new_list = []
        changed = False
        for ins in bb.instructions:
            si = ins.sync_info
            lim = 1
            if si is not None and si.on_wait is not None and len(si.on_wait) > lim:
                waits = list(si.on_wait)
                head, tail = waits[:-lim], waits[-lim:]
                for w in head:
                    nop = mybir.InstNoOp(name=f"waitsplit-{cnt[0]}", ins=[], outs=[])
                    cnt[0] += 1
                    nop.engine = ins.engine
                    nop.sync_info = mybir.SyncInfo(on_wait=[w], on_update=[])
                    nop.bass_nofuse = True
                    try:
                        nc.register_instruction(nop)
                    except Exception:
                        pass
                    new_list.append(nop)
                ins.sync_info = mybir.SyncInfo(
                    on_wait=tail, on_update=list(si.on_update or []))
                changed = True
            new_list.append(ins)
        if changed:
            bb.instructions[:] = new_list


def _get_program():
    global _PROGRAM
    if _PROGRAM is None:
        _PROGRAM = _build_program()
    return _PROGRAM


def _pack_inputs(x_b):
    """x_b: [C1, H*W] fp32 -> xs [128, 2, 6400] bf16."""
    bf16 = ml_dtypes.bfloat16
    xr = x_b.reshape(2, C_, S)            # [t, c, s]
    return np.ascontiguousarray(xr.transpose(1, 0, 2)).astype(bf16)


_IN_NAMES = ('w1t', 'c1sig', 'zb', 'dones', 'cg9', 'wc9', 'a2c2')


def kernel(**inputs):
    from concourse.bass_utils import run_bass_kernel_spmd

    x = np.asarray(inputs['x'], dtype=np.float32)
    cst = _fold_constants(**{k: np.asarray(v, dtype=np.float32)
                             for k, v in inputs.items() if k != 'x'})
    nc = _get_program()
    base = {k: cst[k] for k in _IN_NAMES}
    in_maps = [dict(base, xs=_pack_inputs(x[b].reshape(C1, H * W)))
               for b in range(B)]
    res = run_bass_kernel_spmd(nc, in_maps, list(range(B)))
    out = np.empty((B, C2, H, W), dtype=np.float32)
    for b in range(B):
        ob = res.results[b]['out'].astype(np.float32)     # [128, 2, 6400]
        out[b] = ob.transpose(1, 0, 2).reshape(C2, H, W)
    return out
